# revision 26
# baseline (speedup 1.0000x reference)
"""Causal multi-head attention (B=2, T=2048, D=1024, H=16) on 8 TRN2 NeuronCores.

Sharding: core c = (batch b = c//4, head-group g = c%4). Each core owns 4 heads
(= 256 contiguous dims of D) of one batch: Megatron-style tensor parallelism on
heads x data parallelism on batch. Per-core partial output projections are
summed with chunked on-chip ReduceScatters over each batch's 4 cores; the host
only re-assembles the resulting shards.

Device-side layout choices (host pre-transposes, pure data movement):
  - xT  [D, T]        = x[b].T so projections contract D on the partition dim.
  - qT/kT [256, T]    computed directly transposed (dims on partitions).
  - scoresT[k, q]     = k @ qT -> softmax runs in the k-on-partitions domain,
                        so the AV matmul (lhsT=v, rhs=attnT) needs no T x T
                        transpose anywhere.
  - v_aug [k, 4*65]   v with a ones column appended per head: AV then yields
                        yT' [65, span] whose row 64 is the softmax denominator.
  - softmax: exp(s) without row-max subtraction (scores are O(1): the q,k
    projections are variance-1, scale 1/8 folded into Wq host-side), causal
    tile classification (full-skip / full-keep / diagonal-with-mask-values).
  - normalization: per-span stacked reciprocal on DVE, broadcast across
    partitions via a PE rank-1 outer product, applied during the PSUM->SBUF
    evacuation of yT'.
  - per-q-span pipeline: attention -> normalize -> out-projection -> chunked
    ReduceScatter -> output DMA, so collectives overlap the next span.

All matmuls run as float32r (fp32 storage, TF32-like internal rounding, full
PE rate).
"""

import os
import numpy as np
import ml_dtypes

BF16 = ml_dtypes.bfloat16

B, T, D, H = 2, 2048, 1024, 16
HD = D // H                     # 64
NCORES = 8
GROUPS = 4                      # cores per batch (tensor-parallel degree)
HL = H // GROUPS                # heads per core = 4
DL = D // GROUPS                # dims per core = 256
SP = 512                        # free-dim span per matmul (one PSUM bank, fp32)
QS = T // SP                    # 4 q spans
KT = T // 128                   # 16 k tiles
RS_ROWS = T // GROUPS           # 512 rows per ReduceScatter chunk
SCALE = HD ** -0.5

_CACHE = {}


def _build_program():
    import concourse.bass as bass  # noqa: F401  (registers bass machinery)
    import concourse.tile as tile
    from concourse import bacc, mybir

    f32 = mybir.dt.float32
    f32r = mybir.dt.float32r
    bf16 = mybir.dt.bfloat16
    Exp = mybir.ActivationFunctionType.Exp
    Identity = mybir.ActivationFunctionType.Identity

    nc = bacc.Bacc("TRN2", target_bir_lowering=False, debug=False,
                   num_devices=NCORES)

    xT = nc.dram_tensor("xT", [D, T], f32r, kind="ExternalInput")
    wqT = nc.dram_tensor("wqT", [D, DL], f32r, kind="ExternalInput")
    wkT = nc.dram_tensor("wkT", [D, DL], f32r, kind="ExternalInput")
    wvT = nc.dram_tensor("wvT", [D, DL], f32r, kind="ExternalInput")
    woT = nc.dram_tensor("woT", [DL, D], bf16, kind="ExternalInput")
    bqP = nc.dram_tensor("bqP", [128, 2], f32, kind="ExternalInput")
    bkP = nc.dram_tensor("bkP", [128, 2], f32, kind="ExternalInput")
    bv = nc.dram_tensor("bv", [1, DL], f32r, kind="ExternalInput")
    bo = nc.dram_tensor("bo", [1, D], bf16, kind="ExternalInput")
    maskd = nc.dram_tensor("maskd", [KT, 128, SP], bf16, kind="ExternalInput")
    onesd = nc.dram_tensor("onesd", [128, SP], f32r, kind="ExternalInput")
    onesb = nc.dram_tensor("onesb", [128, SP], bf16, kind="ExternalInput")
    out_ext = nc.dram_tensor("out", [QS, 128, D], f32, kind="ExternalOutput")

    with tile.TileContext(nc) as tc:
        with tc.tile_pool(name="main", bufs=1) as main, \
             tc.tile_pool(name="dram", bufs=1, space="DRAM") as dram:
            qT_s = main.tile([128, 2, T], f32r)
            kT_s = main.tile([128, 2, T], f32r)
            v_s = main.tile([128, KT, HL * 65], bf16)
            yT_s = main.tile([128, 2, T], bf16)
            woT_s = main.tile([128, 2, D], bf16)
            bq_s = main.tile([128, 2], f32)
            bk_s = main.tile([128, 2], f32)
            bv_s = main.tile([1, DL], f32r)
            bo_s = main.tile([1, D], bf16)
            ones_s = main.tile([128, SP], f32r)
            onesb_s = main.tile([128, SP], bf16)
            maskd_s = main.tile([128, KT, SP], bf16)

            # one partial/rs tile pair per q-span: avoids false DRAM-tile
            # dependencies between a span's ReduceScatter and the next
            # span's out-projection DMAs
            partials = [dram.tile([RS_ROWS, D], f32, name=f"partial{i}")
                        for i in range(QS)]
            rs_outs = [dram.tile([128, D], f32, name=f"rsout{i}")
                       for i in range(QS)]

            # tiny high-priority loads on the sync queue
            nc.sync.dma_start(out=bq_s, in_=bqP[:])
            nc.sync.dma_start(out=bk_s, in_=bkP[:])
            # small loads on the scalar queue
            nc.scalar.dma_start(out=ones_s, in_=onesd[:])
            nc.scalar.dma_start(out=onesb_s, in_=onesb[:])
            nc.scalar.dma_start(out=bv_s, in_=bv[:])
            nc.scalar.dma_start(out=bo_s, in_=bo[:])
            v_cols = v_s.rearrange("p k (h u) -> p k h u", u=65)[:, :, :, 64:65]
            nc.scalar.dma_start(
                out=v_cols, in_=onesb[:, 0:KT * HL].rearrange(
                    "p (k h u) -> p k h u", h=HL, u=1))

            # ---------------- phase 1: projections ----------------
            with tc.tile_pool(name="proj", bufs=1) as proj, \
                 tc.tile_pool(name="pj_psum", bufs=3, space="PSUM") as pj_psum:
                xt_s = proj.tile([128, 8, T], f32r)
                wq_s = proj.tile([128, 8, DL], f32r)
                wk_s = proj.tile([128, 8, DL], f32r)
                wv_s = proj.tile([128, 8, DL], f32r)

                # critical path first: wq then the x chunks (split across the
                # sync and gpsimd queues); wk/wv follow behind x on gpsimd
                wq_r = wqT[:].rearrange("(c p) n -> c p n", p=128)
                for c in range(8):
                    nc.sync.dma_start(out=wq_s[:, c, :], in_=wq_r[c])
                xT_r = xT[:].rearrange("(c p) t -> c p t", p=128)
                for c in range(8):
                    eng = nc.sync if c % 2 == 0 else nc.gpsimd
                    eng.dma_start(out=xt_s[:, c, :], in_=xT_r[c])
                for w_s, w_d in ((wk_s, wkT), (wv_s, wvT)):
                    w_r = w_d[:].rearrange("(c p) n -> c p n", p=128)
                    for c in range(8):
                        nc.gpsimd.dma_start(out=w_s[:, c, :], in_=w_r[c])
                # bulk loads not needed until later: separate queues
                for i in range(KT):
                    nc.scalar.dma_start(out=maskd_s[:, i, :], in_=maskd[i])
                woT_r = woT[:].rearrange("(c p) n -> c p n", p=128)
                for c in range(2):
                    nc.scalar.dma_start(out=woT_s[:, c, :], in_=woT_r[c])

                # qT / kT: out[dims-chunk, t-span]; bias added during the
                # PSUM->SBUF evacuation (per-partition scalar)
                for w_s, b_s, dst, use_act in ((wq_s, bq_s, qT_s, True),
                                               (wk_s, bk_s, kT_s, False)):
                    for mc in range(2):
                        for s in range(QS):
                            ps = pj_psum.tile([128, SP], f32, tag="pj")
                            for kc in range(8):
                                nc.tensor.matmul(
                                    ps,
                                    lhsT=w_s[:, kc, mc * 128:(mc + 1) * 128],
                                    rhs=xt_s[:, kc, s * SP:(s + 1) * SP],
                                    start=(kc == 0), stop=(kc == 7))
                            dstv = dst[:, mc, s * SP:(s + 1) * SP]
                            if use_act:
                                nc.scalar.activation(
                                    dstv, ps, Identity,
                                    bias=b_s[:, mc:mc + 1])
                            else:
                                nc.vector.tensor_scalar_add(
                                    dstv, ps, b_s[:, mc:mc + 1])

                # v: natural layout; bias via rank-1 matmul (free-dim bias)
                for mt in range(KT):
                    ps = pj_psum.tile([128, DL], f32, tag="pjv")
                    for kc in range(8):
                        nc.tensor.matmul(
                            ps,
                            lhsT=xt_s[:, kc, mt * 128:(mt + 1) * 128],
                            rhs=wv_s[:, kc, :],
                            start=(kc == 0), stop=False)
                    nc.tensor.matmul(ps, lhsT=ones_s[0:1, 0:128], rhs=bv_s,
                                     start=False, stop=True)
                    nc.vector.tensor_copy(
                        v_s[:, mt, :].rearrange(
                            "p (h d) -> p h d", d=65)[:, :, 0:64],
                        ps.rearrange("p (h d) -> p h d", d=64))

            # ---- phase 2: per-span attention + outproj + chunked RS ----
            with tc.tile_pool(name="attn_t", bufs=3) as attn_t, \
                 tc.tile_pool(name="nrm", bufs=2) as nrm, \
                 tc.tile_pool(name="op_sb", bufs=4) as op_sb, \
                 tc.tile_pool(name="sc_psum", bufs=2, space="PSUM") as sc_psum, \
                 tc.tile_pool(name="av_psum", bufs=4, space="PSUM") as av_psum, \
                 tc.tile_pool(name="op_psum", bufs=2, space="PSUM") as op_psum:
                for qs in range(QS):
                    # denominator rows live at partitions 0/32/64/96 (engine
                    # APs must start 32-aligned); memset keeps the unused
                    # rows finite for the reciprocal
                    den_stack = nrm.tile([97, SP], f32, tag="den")
                    nc.vector.memset(den_stack, 1.0)
                    yT_pss = []
                    nkt = 4 * qs + 4  # causal: later k tiles are all-masked
                    for h in range(HL):
                        mc, r0 = divmod(h, 2)
                        r0 *= 64
                        qv = qT_s[r0:r0 + 64, mc, qs * SP:(qs + 1) * SP]
                        yT_ps = av_psum.tile([65, SP], f32, tag="av")
                        yT_pss.append(yT_ps)
                        for kt in range(nkt):
                            sc = sc_psum.tile([128, SP], f32, tag="sc")
                            nc.tensor.matmul(
                                sc,
                                lhsT=kT_s[r0:r0 + 64, mc,
                                          kt * 128:(kt + 1) * 128],
                                rhs=qv, start=True, stop=True)
                            at = attn_t.tile([128, SP], bf16, tag="at")
                            nc.scalar.activation(at, sc, Exp)
                            if kt >= 4 * qs:  # diagonal tile: apply mask
                                nc.vector.tensor_mul(at, at, maskd_s[:, kt, :])
                            nc.tensor.matmul(
                                yT_ps, lhsT=v_s[:, kt, h * 65:(h + 1) * 65],
                                rhs=at, start=(kt == 0), stop=(kt == nkt - 1))
                        nc.vector.tensor_copy(den_stack[32 * h:32 * h + 1, :],
                                              yT_ps[64:65, :])
                    rec_f = nrm.tile([97, SP], f32, tag="recf")
                    nc.vector.reciprocal(rec_f, den_stack)
                    for h in range(HL):
                        mc, r0 = divmod(h, 2)
                        r0 *= 64
                        # PE operands must start at partition 0: copy the
                        # head's reciprocal row into its own (f32r) tile
                        rec_h = nrm.tile([1, SP], bf16, tag="rech", bufs=4)
                        nc.vector.tensor_copy(rec_h, rec_f[32 * h:32 * h + 1, :])
                        # broadcast 1/denom across 64 partitions on the PE
                        rb = sc_psum.tile([64, SP], f32, tag="sc")
                        nc.tensor.matmul(rb, lhsT=onesb_s[0:1, 0:64],
                                         rhs=rec_h,
                                         start=True, stop=True)
                        # DVE reads at most one PSUM input: evacuate yT'
                        # first (ACT), then scale in place
                        yv = yT_s[r0:r0 + 64, mc, qs * SP:(qs + 1) * SP]
                        nc.scalar.copy(yv, yT_pss[h][0:64, :])
                        nc.vector.tensor_mul(yv, yv, rb)
                    # out-projection for this span's 4 q-tiles
                    for qt in range(4 * qs, 4 * qs + 4):
                        for ns in range(2):
                            po = op_psum.tile([128, SP], f32, tag="op")
                            for kc in range(2):
                                nc.tensor.matmul(
                                    po,
                                    lhsT=yT_s[:, kc, qt * 128:(qt + 1) * 128],
                                    rhs=woT_s[:, kc, ns * SP:(ns + 1) * SP],
                                    start=(kc == 0), stop=False)
                            nc.tensor.matmul(
                                po, lhsT=onesb_s[0:1, 0:128],
                                rhs=bo_s[:, ns * SP:(ns + 1) * SP],
                                start=False, stop=True)
                            ob = op_sb.tile([128, SP], f32, tag="ob")
                            if ns == 0:
                                nc.vector.tensor_copy(ob, po)
                            else:
                                nc.scalar.copy(ob, po)
                            nc.sync.dma_start(
                                out=partials[qs][
                                    (qt - 4 * qs) * 128:(qt - 4 * qs + 1) * 128,
                                    ns * SP:(ns + 1) * SP],
                                in_=ob)
                    # chunked ReduceScatter of this span's 512 rows; the
                    # last span goes in two halves so the tail is shorter
                    halves = 2 if qs == QS - 1 else 1
                    hr = RS_ROWS // halves
                    for hf in range(halves):
                        nc.gpsimd.collective_compute(
                            "ReduceScatter", mybir.AluOpType.add,
                            replica_groups=[[0, 1, 2, 3], [4, 5, 6, 7]],
                            ins=[partials[qs][hf * hr:(hf + 1) * hr, :].opt()],
                            outs=[rs_outs[qs][hf * (128 // halves):
                                              (hf + 1) * (128 // halves),
                                              :].opt()])
                        nc.sync.dma_start(
                            out=out_ext[qs, hf * (128 // halves):
                                        (hf + 1) * (128 // halves), :],
                            in_=rs_outs[qs][hf * (128 // halves):
                                            (hf + 1) * (128 // halves), :])

    nc.compile()
    return nc


def _get_program():
    if "nc" not in _CACHE:
        _CACHE["nc"] = _build_program()
    return _CACHE["nc"]


def _make_in_maps(x, mask, Wq, bq, Wk, bk, Wv, bv, Wo, bo):
    x = np.asarray(x, np.float32)
    mask = np.asarray(mask, bool)
    Wq = np.asarray(Wq, np.float32)
    Wk = np.asarray(Wk, np.float32)
    Wv = np.asarray(Wv, np.float32)
    Wo = np.asarray(Wo, np.float32)
    bq = np.asarray(bq, np.float32)
    bk = np.asarray(bk, np.float32)
    bv = np.asarray(bv, np.float32)
    bo = np.asarray(bo, np.float32)

    zeros_bo = np.zeros((1, D), np.float32)
    in_maps = []
    per_batch = {}
    for b in range(B):
        xTb = np.ascontiguousarray(x[b].T)
        # diagonal mask tiles of mask[b,0].T: index qs*4+j holds
        # maskT[128*(4qs+j) : +128, 512*qs : +512]
        mT = mask[b, 0].T
        md = np.empty((KT, 128, SP), np.float32)
        for qs in range(QS):
            for j in range(4):
                kt = 4 * qs + j
                md[kt] = mT[kt * 128:(kt + 1) * 128,
                            qs * SP:(qs + 1) * SP].astype(np.float32)
        per_batch[b] = (xTb, md)
    for c in range(NCORES):
        b, g = divmod(c, GROUPS)
        sl = slice(g * DL, (g + 1) * DL)
        xTb, md = per_batch[b]
        in_maps.append({
            "xT": xTb,
            "wqT": np.ascontiguousarray((Wq[sl] * SCALE).T),
            "wkT": np.ascontiguousarray(Wk[sl].T),
            "wvT": np.ascontiguousarray(Wv[sl].T),
            "woT": np.ascontiguousarray(Wo[:, sl].T).astype(BF16),
            "bqP": np.ascontiguousarray((bq[sl] * SCALE).reshape(2, 128).T),
            "bkP": np.ascontiguousarray(bk[sl].reshape(2, 128).T),
            "bv": bv[sl].reshape(1, DL),
            "bo": (bo.reshape(1, D) if g == 0 else zeros_bo).astype(BF16),
            "maskd": md.astype(BF16),
            "onesd": np.ones((128, SP), np.float32),
            "onesb": np.ones((128, SP), BF16),
        })
    return in_maps


def _capture_profile(nc, in_maps, tmpdir):
    """Run with NTFF capture and process the profile ourselves (the stock
    trace path can't handle the duplicate-executable NTFFs the axon relay
    produces). Returns (results, exec_time_ns|None)."""
    import glob
    import json
    import re
    import subprocess
    from trn_agent_boot.trn_boot import _ntff_profile_via_ctypes
    from concourse import bass2jax

    hook = _ntff_profile_via_ctypes("/opt/axon/libaxon_pjrt.so")
    if hook is None:
        raise RuntimeError("libaxon_pjrt.so lacks NTFF profile symbols")
    os.makedirs(tmpdir, exist_ok=True)
    with hook(tmpdir, [0]):
        results = bass2jax.run_bass_via_pjrt(nc, in_maps, n_cores=NCORES)

    # group NTFF/NEFF pairs by executable id; use the newest executable
    ntffs = glob.glob(os.path.join(tmpdir, "*_body*-device*.ntff"))
    best, best_id = None, -1
    for f in ntffs:
        m = re.search(r"executable(\d+)-device000000", f)
        if m and int(m.group(1)) > best_id:
            best_id, best = int(m.group(1)), f
    if best is None:
        raise RuntimeError(f"no NTFF produced in {tmpdir}")
    neff = re.sub(r"-device\d+-execution-\d+\.ntff$", ".neff", best)
    out_json = os.path.join(tmpdir, "prof.json")
    subprocess.check_call(
        ["neuron-profile", "view", "--ignore-nc-buf-usage", "-s", best,
         "-n", neff, "--output-format=json", f"--output-file={out_json}"],
        cwd=tmpdir)
    summary = json.load(open(out_json))["summary"][0]
    return results, int(summary["total_time"] * 1e9)


def kernel(x, mask, Wq, bq, Wk, bk, Wv, bv, Wo, bo):
    from concourse import bass_utils

    in_maps = _make_in_maps(x, mask, Wq, bq, Wk, bk, Wv, bv, Wo, bo)
    nc = _get_program()

    trace = bool(int(os.environ.get("MHA_TRACE", "0")))
    tmpdir = os.environ.get("MHA_TRACE_DIR") or None
    results = None
    if trace and tmpdir:
        try:
            results, exec_ns = _capture_profile(nc, in_maps, tmpdir)
            _CACHE["last_exec_time_ns"] = exec_ns
        except Exception as e:  # profiling is best-effort
            print(f"profiling unavailable: {type(e).__name__}: {e}")
            results = None
    if results is None:
        results = bass_utils.run_bass_kernel_spmd(
            nc, in_maps, core_ids=list(range(NCORES))).results
        _CACHE.setdefault("last_exec_time_ns", None)

    out = np.empty((B, T, D), np.float32)
    for c in range(NCORES):
        b, rk = divmod(c, GROUPS)
        o = results[c]["out"]
        for qs in range(QS):
            if qs == QS - 1:  # final span was reduce-scattered in two halves
                for hf in range(2):
                    lo = qs * RS_ROWS + hf * (RS_ROWS // 2) + rk * 64
                    out[b, lo:lo + 64] = o[qs, hf * 64:(hf + 1) * 64]
            else:
                out[b, qs * RS_ROWS + rk * 128:
                       qs * RS_ROWS + (rk + 1) * 128] = o[qs]
    return out
